# revision 1
# baseline (speedup 1.0000x reference)
"""Trainium2 Bass kernel for a 3-layer GCN (KnowledgeGraphGNN).

Reference computation (per layer i):
    support = h @ W[i]                       # [N, 128]
    h = relu(adj @ support + b[i])           # [N, 128]
    h = BN(h) (training stats, biased var)   # [N, 128]
final:  out = h @ Wout + bout                # [N, 64]

Sharding: nodes row-partitioned 8 ways.  Each core keeps its adj^T column
block [N, R] (bf16) resident in SBUF and the *full* support matrix S [N, 128]
(bf16) as 64 stationary k-tiles.  The aggregate matmul is computed in
transposed space:  h_c^T [128, R] = S^T @ adjT_c, accumulated over 64 k-tiles
on the PE.  BN stats are a [128, 2] AllReduce; the next layer's support rows
are computed locally and AllGathered (bf16, 256 KB/rank).
"""

import numpy as np
import ml_dtypes

BF16 = ml_dtypes.bfloat16

N = 8192          # nodes
DH = 128          # hidden dim (= partition count)
DOUT = 64
NC = 8            # cores
R = N // NC       # rows per core = 1024
KT = N // 128     # contraction tiles = 64
RT = R // 128     # node tiles per core = 8
NLAYERS = 3
EPS = 1e-5

_cache = {}


def _build_module():
    from concourse import bacc, tile
    import concourse.mybir as mybir

    f32 = mybir.dt.float32
    bf16 = mybir.dt.bfloat16
    AF = mybir.ActivationFunctionType

    nc = bacc.Bacc(None, target_bir_lowering=False, num_devices=NC)

    # ---- kernel I/O --------------------------------------------------------
    adjt = nc.dram_tensor("adjt", [N, R], bf16, kind="ExternalInput")
    xt = nc.dram_tensor("xt", [128, N], bf16, kind="ExternalInput")
    w0 = nc.dram_tensor("w0", [128, 128], bf16, kind="ExternalInput")
    w1 = nc.dram_tensor("w1", [128, 128], bf16, kind="ExternalInput")
    w2 = nc.dram_tensor("w2", [128, 128], bf16, kind="ExternalInput")
    wout = nc.dram_tensor("wout", [128, DOUT], bf16, kind="ExternalInput")
    boutb = nc.dram_tensor("boutb", [1, DOUT], bf16, kind="ExternalInput")
    biasd = nc.dram_tensor("biasd", [NLAYERS, 128, 1], f32, kind="ExternalInput")
    gammad = nc.dram_tensor("gammad", [NLAYERS, 128, 1], f32, kind="ExternalInput")
    betad = nc.dram_tensor("betad", [NLAYERS, 128, 1], f32, kind="ExternalInput")
    out = nc.dram_tensor("out", [R, DOUT], f32, kind="ExternalOutput")

    rg = [list(range(NC))]

    with tile.TileContext(nc) as tc:
        with (
            tc.tile_pool(name="const", bufs=1) as const,
            tc.tile_pool(name="adjp", bufs=1) as adjp,
            tc.tile_pool(name="sp", bufs=1) as sp,
            tc.tile_pool(name="work", bufs=1) as work,
            tc.tile_pool(name="psum", bufs=1, space="PSUM") as psum,
            tc.tile_pool(name="psmall", bufs=4, space="PSUM") as psmall,
            tc.tile_pool(name="dram", bufs=1, space="DRAM") as dram,
        ):
            # ---- constants -------------------------------------------------
            w_sb = []
            for i, wd in enumerate((w0, w1, w2)):
                t = const.tile([128, 128], bf16, name=f"w{i}_sb", tag=f"w{i}_sb")
                nc.sync.dma_start(t[:], wd[:])
                w_sb.append(t)
            wout_sb = const.tile([128, DOUT], bf16, name="wout_sb")
            nc.sync.dma_start(wout_sb[:], wout[:])
            boutb_sb = const.tile([1, DOUT], bf16, name="boutb_sb")
            nc.sync.dma_start(boutb_sb[:], boutb[:])
            ones_sb = const.tile([1, 128], bf16, name="ones_sb")
            nc.vector.memset(ones_sb[:], 1.0)
            bias_sb = const.tile([128, NLAYERS], f32, name="bias_sb")
            gamma_sb = const.tile([128, NLAYERS], f32, name="gamma_sb")
            beta_sb = const.tile([128, NLAYERS], f32, name="beta_sb")
            for i in range(NLAYERS):
                nc.sync.dma_start(bias_sb[:, i : i + 1], biasd[i])
                nc.sync.dma_start(gamma_sb[:, i : i + 1], gammad[i])
                nc.sync.dma_start(beta_sb[:, i : i + 1], betad[i])

            xt_sb = const.tile([128, N], bf16, name="xt_sb")
            nc.sync.dma_start(xt_sb[:], xt[:])

            # ---- adj^T resident in SBUF (64 k-tiles, bf16) -----------------
            adj_t = []
            for k in range(KT):
                t = adjp.tile([128, R], bf16, name=f"adj_{k}", tag=f"adj_{k}")
                nc.sync.dma_start(t[:], adjt[k * 128 : (k + 1) * 128, :])
                adj_t.append(t)

            # ---- S tiles (full support matrix, natural layout k-tiles) ----
            s_t = [
                sp.tile([128, 128], bf16, name=f"s_{k}", tag=f"s_{k}")
                for k in range(KT)
            ]

            # Layer 0 support: S0 = x @ W0, computed fully on every core.
            for k in range(KT):
                ps0 = psmall.tile([128, 128], f32, name="ps0", tag="pg")
                nc.tensor.matmul(
                    ps0[:], xt_sb[:, k * 128 : (k + 1) * 128], w_sb[0][:]
                )
                nc.vector.tensor_copy(s_t[k][:], ps0[:])

            for layer in range(NLAYERS):
                # ---- A: h^T [128, R] = S^T @ adjT_c (PE, k-contiguous) ----
                ph = psum.tile([128, R], f32, name="ph", tag="ph")
                for nch in range(R // 512):
                    lo = nch * 512
                    for k in range(KT):
                        nc.tensor.matmul(
                            ph[:, lo : lo + 512],
                            s_t[k][:],
                            adj_t[k][:, lo : lo + 512],
                            start=(k == 0),
                            stop=(k == KT - 1),
                        )

                # ---- B: z = relu(h + b), partial sum ----------------------
                z = work.tile([128, R], f32, name="z", tag="z")
                stats = work.tile([128, 2], f32, name="stats", tag="stats", bufs=2)
                nc.scalar.activation(
                    z[:],
                    ph[:],
                    AF.Relu,
                    bias=bias_sb[:, layer : layer + 1],
                    scale=1.0,
                    accum_out=stats[:, 0:1],
                )
                # ---- C: partial sum of z^2 (square dumped into psum) ------
                nc.scalar.activation(
                    ph[:], z[:], AF.Square, accum_out=stats[:, 1:2]
                )

                # ---- D: AllReduce of [128, 2] stats -----------------------
                cc_in = dram.tile(
                    [128, 2], f32, name=f"ccin{layer}", tag=f"ccin{layer}"
                )
                cc_out = dram.tile(
                    [128, 2], f32, name=f"ccout{layer}", tag=f"ccout{layer}",
                    addr_space="Shared",
                )
                nc.sync.dma_start(cc_in[:], stats[:])
                nc.gpsimd.collective_compute(
                    "AllReduce",
                    mybir.AluOpType.add,
                    replica_groups=rg,
                    ins=[cc_in.opt()],
                    outs=[cc_out.opt()],
                )
                gst = work.tile([128, 2], f32, name="gst", tag="gst", bufs=2)
                nc.sync.dma_start(gst[:], cc_out[:])

                # ---- E: BN affine coefficients [128, 1] -------------------
                mu = work.tile([128, 1], f32, name="mu", tag="mu")
                ex2 = work.tile([128, 1], f32, name="ex2", tag="ex2")
                var = work.tile([128, 1], f32, name="var", tag="var")
                sd = work.tile([128, 1], f32, name="sd", tag="sd")
                inv = work.tile([128, 1], f32, name="inv", tag="inv")
                aco = work.tile([128, 1], f32, name="aco", tag="aco")
                bsh = work.tile([128, 1], f32, name="bsh", tag="bsh")
                nc.vector.tensor_scalar_mul(mu[:], gst[:, 0:1], 1.0 / N)
                nc.vector.tensor_scalar_mul(ex2[:], gst[:, 1:2], 1.0 / N)
                nc.vector.tensor_mul(var[:], mu[:], mu[:])
                nc.vector.tensor_sub(var[:], ex2[:], var[:])
                nc.vector.tensor_scalar_add(var[:], var[:], EPS)
                nc.scalar.sqrt(sd[:], var[:])
                nc.vector.reciprocal(inv[:], sd[:])
                nc.vector.tensor_mul(aco[:], gamma_sb[:, layer : layer + 1], inv[:])
                nc.vector.tensor_mul(bsh[:], mu[:], aco[:])
                nc.vector.tensor_sub(bsh[:], beta_sb[:, layer : layer + 1], bsh[:])

                # ---- F: zaff = z * a + b' (BN affine), bf16 ---------------
                zaff = work.tile([128, R], bf16, name="zaff", tag="zaff")
                nc.scalar.activation(
                    zaff[:], z[:], AF.Identity, bias=bsh[:], scale=aco[:]
                )

                if layer < NLAYERS - 1:
                    # ---- G: S'_c rows = BN(z)_c @ W[layer+1] --------------
                    spr = work.tile([128, R], bf16, name="spr", tag="spr")
                    for t in range(RT):
                        pg = psmall.tile([128, 128], f32, name="pg", tag="pg")
                        nc.tensor.matmul(
                            pg[:],
                            zaff[:, t * 128 : (t + 1) * 128],
                            w_sb[layer + 1][:],
                        )
                        nc.vector.tensor_copy(
                            spr[:, t * 128 : (t + 1) * 128], pg[:]
                        )
                    # ---- H: AllGather S' and reload stationary tiles ------
                    agi = dram.tile(
                        [R, 128], bf16, name=f"agi{layer}", tag=f"agi{layer}"
                    )
                    ago = dram.tile(
                        [N, 128], bf16, name=f"ago{layer}", tag=f"ago{layer}",
                        addr_space="Shared",
                    )
                    for t in range(RT):
                        nc.sync.dma_start(
                            agi[t * 128 : (t + 1) * 128, :],
                            spr[:, t * 128 : (t + 1) * 128],
                        )
                    nc.gpsimd.collective_compute(
                        "AllGather",
                        mybir.AluOpType.bypass,
                        replica_groups=rg,
                        ins=[agi.opt()],
                        outs=[ago.opt()],
                    )
                    for k in range(KT):
                        nc.sync.dma_start(
                            s_t[k][:], ago[k * 128 : (k + 1) * 128, :]
                        )
                else:
                    # ---- output layer: out_c = BN(z)_c @ Wout + bout ------
                    osb = work.tile([128, RT * DOUT], f32, name="osb", tag="osb")
                    for t in range(RT):
                        po = psmall.tile([128, DOUT], f32, name="po", tag="pg")
                        nc.tensor.matmul(
                            po[:], ones_sb[:], boutb_sb[:],
                            start=True, stop=False,
                        )
                        nc.tensor.matmul(
                            po[:],
                            zaff[:, t * 128 : (t + 1) * 128],
                            wout_sb[:],
                            start=False, stop=True,
                        )
                        nc.vector.tensor_copy(
                            osb[:, t * DOUT : (t + 1) * DOUT], po[:]
                        )
                        nc.sync.dma_start(
                            out[t * 128 : (t + 1) * 128, :],
                            osb[:, t * DOUT : (t + 1) * DOUT],
                        )

    nc.compile()
    return nc


def _get_module():
    if "nc" not in _cache:
        _cache["nc"] = _build_module()
    return _cache["nc"]


def _prep_inputs(inputs):
    """Host-side sharding / layout prep (transpose + bf16 cast + slice)."""
    x = np.asarray(inputs["x"], np.float32)
    adj = np.asarray(inputs["adj"], np.float32)
    xt = np.ascontiguousarray(x.T).astype(BF16)                   # [128, N]
    bias = np.stack(
        [np.asarray(inputs[f"b{i}"], np.float32) for i in range(NLAYERS)]
    ).reshape(NLAYERS, 128, 1)
    gamma = np.stack(
        [np.asarray(inputs[f"g{i}"], np.float32) for i in range(NLAYERS)]
    ).reshape(NLAYERS, 128, 1)
    beta = np.stack(
        [np.asarray(inputs[f"be{i}"], np.float32) for i in range(NLAYERS)]
    ).reshape(NLAYERS, 128, 1)
    common = {
        "xt": xt,
        "w0": np.asarray(inputs["W0"], np.float32).astype(BF16),
        "w1": np.asarray(inputs["W1"], np.float32).astype(BF16),
        "w2": np.asarray(inputs["W2"], np.float32).astype(BF16),
        "wout": np.asarray(inputs["Wout"], np.float32).astype(BF16),
        "boutb": np.asarray(inputs["bout"], np.float32).reshape(1, DOUT).astype(BF16),
        "biasd": bias,
        "gammad": gamma,
        "betad": beta,
    }
    in_maps = []
    for c in range(NC):
        adjt_c = np.ascontiguousarray(
            adj[c * R : (c + 1) * R, :].astype(BF16).T
        )                                                          # [N, R]
        in_maps.append({"adjt": adjt_c, **common})
    return in_maps


def run(inputs, trace=False):
    from concourse.bass_utils import run_bass_kernel_spmd

    nc = _get_module()
    in_maps = _prep_inputs(inputs)
    res = run_bass_kernel_spmd(
        nc, in_maps, core_ids=list(range(NC)), trace=trace
    )
    out = np.concatenate(
        [res.results[c]["out"] for c in range(NC)], axis=0
    ).astype(np.float32)
    return out, res


def kernel(**inputs):
    out, _ = run(inputs, trace=False)
    return out


# revision 4
# speedup vs baseline: 1.3405x; 1.3405x over previous
"""Trainium2 Bass kernel for a 3-layer GCN (KnowledgeGraphGNN).

Reference (per layer i):  h = BN_i(relu(adj @ (h @ W_i) + b_i)),  then
out = h @ Wout + bout.

Sharding: nodes row-partitioned over 8 cores.  Each core keeps its adj^T
column block [N, R] resident in SBUF as fp8 (adj is 0/1 -> exact) and the
full "stationary" activation matrix [N, 128] in bf16 k-tiles.  The
aggregation matmul runs in transposed space: P^T [128, R] = S^T @ adjT_c,
64 k-tiles accumulated on the PE (N=512 moving slices).

Collective structure (one AllGather per hidden layer): the *raw* post-ReLU
activations R_i (bf16, natural layout via PE transposes) are gathered
together with the per-core BN partial sums packed into the same buffer.
The BN affine is folded into the next layer algebraically:

    h_{i+1} = adj @ (BN_i(R_i) @ W) = (adj @ R_i) @ diag(a_i) W  +  d x r_i

with a_i = gamma_i/sigma_i, r_i = (beta_i - mu_i a_i) @ W, and d = adj @ 1
the degree vector (shipped as adjacency metadata).  The rank-1 d x r term
is seeded into PSUM with a K=1 outer-product matmul before the W matmuls
accumulate on top.  The final layer needs only a tiny stats AllGather.
"""

import numpy as np
import ml_dtypes

BF16 = ml_dtypes.bfloat16
FP8 = ml_dtypes.float8_e4m3

N = 8192          # nodes
DH = 128          # hidden dim (= partition count)
DOUT = 64
NC = 8            # cores
R = N // NC       # rows per core = 1024
KT = N // 128     # contraction k-tiles = 64
G = 8             # k-tile groups (8 tiles each)
RT = R // 128     # node tiles per core = 8
NLAYERS = 3
EPS = 1e-5
AGROWS = R + 4    # AG payload rows: R activation rows + 4 rows (=1KB) stats

_cache = {}


def _build_module():
    from concourse import bacc, tile
    import concourse.mybir as mybir

    f32 = mybir.dt.float32
    bf16 = mybir.dt.bfloat16
    fp8 = mybir.dt.float8e4
    AF = mybir.ActivationFunctionType

    nc = bacc.Bacc(None, target_bir_lowering=False, num_devices=NC)

    # ---- kernel I/O --------------------------------------------------------
    adjt = nc.dram_tensor("adjt", [N, R], fp8, kind="ExternalInput")
    xt = nc.dram_tensor("xt", [128, N], bf16, kind="ExternalInput")
    w0 = nc.dram_tensor("w0", [128, 128], bf16, kind="ExternalInput")
    w1 = nc.dram_tensor("w1", [128, 128], bf16, kind="ExternalInput")
    w2 = nc.dram_tensor("w2", [128, 128], bf16, kind="ExternalInput")
    wout = nc.dram_tensor("wout", [128, DOUT], bf16, kind="ExternalInput")
    boutb = nc.dram_tensor("boutb", [1, DOUT], bf16, kind="ExternalInput")
    dd = nc.dram_tensor("dd", [1, R], bf16, kind="ExternalInput")
    idn = nc.dram_tensor("idn", [128, 128], bf16, kind="ExternalInput")
    biasd = nc.dram_tensor("biasd", [NLAYERS, 128, 1], f32, kind="ExternalInput")
    gammad = nc.dram_tensor("gammad", [NLAYERS, 128, 1], f32, kind="ExternalInput")
    betad = nc.dram_tensor("betad", [NLAYERS, 128, 1], f32, kind="ExternalInput")
    out = nc.dram_tensor("out", [R, DOUT], f32, kind="ExternalOutput")

    rg = [list(range(NC))]
    wdram = (w0, w1, w2)

    with tile.TileContext(nc) as tc:
        with (
            tc.tile_pool(name="const", bufs=1) as const,
            tc.tile_pool(name="adjp", bufs=1) as adjp,
            tc.tile_pool(name="sp", bufs=1) as sp,
            tc.tile_pool(name="work", bufs=1) as work,
            tc.tile_pool(name="psA", bufs=1, space="PSUM") as psA,
            tc.tile_pool(name="psH", bufs=1, space="PSUM") as psH,
            tc.tile_pool(name="psS", bufs=2, space="PSUM") as psS,
            tc.tile_pool(name="psT", bufs=2, space="PSUM") as psT,
            tc.tile_pool(name="dram", bufs=1, space="DRAM") as dram,
        ):
            # ---- constants (scalar engine issues these tiny DMAs) ----------
            w_sb = []
            for i in range(NLAYERS):
                t = const.tile([128, 128], bf16, name=f"w{i}_sb", tag=f"w{i}_sb")
                nc.scalar.dma_start(t[:], wdram[i][:])
                w_sb.append(t)
            wout_sb = const.tile([128, DOUT], bf16, name="wout_sb")
            nc.scalar.dma_start(wout_sb[:], wout[:])
            boutb_sb = const.tile([1, DOUT], bf16, name="boutb_sb")
            nc.scalar.dma_start(boutb_sb[:], boutb[:])
            ones_sb = const.tile([1, 128], bf16, name="ones_sb")
            nc.vector.memset(ones_sb[:], 1.0)
            d_sb = const.tile([1, R], bf16, name="d_sb")
            nc.scalar.dma_start(d_sb[:], dd[:])
            idn_sb = const.tile([128, 128], bf16, name="idn_sb")
            nc.scalar.dma_start(idn_sb[:], idn[:])
            bias_sb = const.tile([128, NLAYERS], f32, name="bias_sb")
            gamma_sb = const.tile([128, NLAYERS], f32, name="gamma_sb")
            beta_sb = const.tile([128, NLAYERS], f32, name="beta_sb")
            for i in range(NLAYERS):
                nc.scalar.dma_start(bias_sb[:, i : i + 1], biasd[i])
                nc.scalar.dma_start(gamma_sb[:, i : i + 1], gammad[i])
                nc.scalar.dma_start(beta_sb[:, i : i + 1], betad[i])

            # x^T first so the layer-0 stationary build can start immediately
            xt_sb = const.tile([128, N], bf16, name="xt_sb")
            nc.sync.dma_start(xt_sb[:], xt[:])

            # ---- adj^T resident in SBUF: 8 group tiles, 1 DMA each ---------
            adj_g = []
            for g in range(G):
                t = adjp.tile([128, 8, R], fp8, name=f"adj_{g}", tag=f"adj_{g}")
                src = adjt[g * 1024 : (g + 1) * 1024, :].rearrange(
                    "(k p) c -> p k c", p=128
                )
                nc.gpsimd.dma_start(t[:], src)
                adj_g.append(t)

            def adj_mv(k, lo, size):
                g, sub = divmod(k, 8)
                return adj_g[g][:, sub, lo : lo + size]

            # ---- stationary activation tiles (8 groups of 8 k-tiles) -------
            s_g = [
                sp.tile([128, 8, 128], bf16, name=f"s_{g}", tag=f"s_{g}")
                for g in range(G)
            ]

            def s_tile(k):
                g, sub = divmod(k, 8)
                return s_g[g][:, sub, :]

            # Layer 0 stationary: S0 = x @ W0, built locally on every core.
            for k in range(KT):
                ps0 = psS.tile([128, 128], f32, name="ps0", tag="psS")
                nc.tensor.matmul(ps0[:], xt_sb[:, k * 128 : (k + 1) * 128], w_sb[0][:])
                nc.vector.tensor_copy(s_tile(k), ps0[:])

            # per-layer DRAM comm tiles
            agi = [
                dram.tile([AGROWS, 128], bf16, name=f"agi{i}", tag=f"agi{i}")
                for i in range(2)
            ]
            ago = [
                dram.tile(
                    [NC * AGROWS, 128], bf16, name=f"ago{i}", tag=f"ago{i}",
                    addr_space="Shared",
                )
                for i in range(2)
            ]
            agi2 = dram.tile([4, 128], bf16, name="agi2", tag="agi2")
            ago2 = dram.tile([32, 128], bf16, name="ago2", tag="ago2",
                             addr_space="Shared")

            gstats = None  # SBUF tile holding the 8 gathered stat blocks

            for i in range(NLAYERS):
                # ---- A: P^T [128, R] = S^T @ adjT_c  (64 k-tiles, N=512) --
                if i == 0:
                    ph = psH.tile([128, R], f32, name="ph", tag="ph")
                    for nch in range(R // 512):
                        lo = nch * 512
                        for k in range(KT):
                            nc.tensor.matmul(
                                ph[:, lo : lo + 512],
                                s_tile(k),
                                adj_mv(k, lo, 512),
                                start=(k == 0),
                                stop=(k == KT - 1),
                            )
                else:
                    pa = psA.tile([128, R], f32, name="pa", tag="pa")
                    for nch in range(R // 512):
                        lo = nch * 512
                        for k in range(KT):
                            nc.tensor.matmul(
                                pa[:, lo : lo + 512],
                                s_tile(k),
                                adj_mv(k, lo, 512),
                                start=(k == 0),
                                stop=(k == KT - 1),
                            )
                    pm = work.tile([128, R], bf16, name="pm", tag="pm")
                    nc.vector.tensor_copy(pm[:], pa[:])

                    # stats of layer i-1 arrived inside AG i-1: combine them.
                    gsc = work.tile([128, 8], f32, name="gsc", tag="gsc")
                    st2 = work.tile([128, 2], f32, name="st2", tag="st2")
                    nc.vector.tensor_add(gsc[:], gstats[:, 0:8], gstats[:, 8:16])
                    nc.vector.tensor_add(gsc[:, 0:4], gsc[:, 0:4], gsc[:, 4:8])
                    nc.vector.tensor_add(st2[:], gsc[:, 0:2], gsc[:, 2:4])
                    # mu, var, a = gamma/sigma, c = beta - mu*a
                    mu = work.tile([128, 1], f32, name="mu", tag="mu")
                    ex2 = work.tile([128, 1], f32, name="ex2", tag="ex2")
                    var = work.tile([128, 1], f32, name="var", tag="var")
                    sd = work.tile([128, 1], f32, name="sd", tag="sd")
                    inv = work.tile([128, 1], f32, name="inv", tag="inv")
                    aco = work.tile([128, 1], f32, name="aco", tag="aco")
                    cco = work.tile([128, 1], f32, name="cco", tag="cco")
                    ccb = work.tile([128, 1], bf16, name="ccb", tag="ccb")
                    nc.vector.tensor_scalar_mul(mu[:], st2[:, 0:1], 1.0 / N)
                    nc.vector.tensor_scalar_mul(ex2[:], st2[:, 1:2], 1.0 / N)
                    nc.vector.tensor_mul(var[:], mu[:], mu[:])
                    nc.vector.tensor_sub(var[:], ex2[:], var[:])
                    nc.vector.tensor_scalar_add(var[:], var[:], EPS)
                    nc.scalar.sqrt(sd[:], var[:])
                    nc.vector.reciprocal(inv[:], sd[:])
                    nc.vector.tensor_mul(aco[:], gamma_sb[:, i - 1 : i], inv[:])
                    nc.vector.tensor_mul(cco[:], mu[:], aco[:])
                    nc.vector.tensor_sub(cco[:], beta_sb[:, i - 1 : i], cco[:])
                    nc.vector.tensor_copy(ccb[:], cco[:])
                    # Wa = diag(a) @ W_i  (bf16), r = c @ W_i  (bf16 row)
                    wa = work.tile([128, 128], bf16, name="wa", tag="wa")
                    nc.scalar.activation(wa[:], w_sb[i][:], AF.Copy, scale=aco[:])
                    pr = psS.tile([1, 128], f32, name="pr", tag="psS")
                    nc.tensor.matmul(pr[:], ccb[:], w_sb[i][:])
                    rrow = work.tile([1, 128], bf16, name="rrow", tag="rrow")
                    nc.vector.tensor_copy(rrow[:], pr[:])

                    # ---- transform: ph = Wa^T @ Pm + outer(r, d) ----------
                    ph = psH.tile([128, R], f32, name="ph", tag="ph")
                    for nch in range(R // 512):
                        lo = nch * 512
                        nc.tensor.matmul(
                            ph[:, lo : lo + 512],
                            rrow[:],
                            d_sb[:, lo : lo + 512],
                            start=True, stop=False,
                        )
                        nc.tensor.matmul(
                            ph[:, lo : lo + 512],
                            wa[:],
                            pm[:, lo : lo + 512],
                            start=False, stop=True,
                        )

                # ---- B/C: zb = relu(ph + b_i) (bf16) + partial stats ------
                zb = work.tile([128, R], bf16, name="zb", tag="zb")
                sq = work.tile([128, R], bf16, name="sq", tag="sq")
                st4 = work.tile([128, 4], f32, name="st4", tag="st4")
                for c in range(2):
                    lo = c * 512
                    nc.scalar.activation(
                        zb[:, lo : lo + 512],
                        ph[:, lo : lo + 512],
                        AF.Relu,
                        bias=bias_sb[:, i : i + 1],
                        scale=1.0,
                        accum_out=st4[:, 2 * c : 2 * c + 1],
                    )
                    nc.scalar.activation(
                        sq[:, lo : lo + 512],
                        zb[:, lo : lo + 512],
                        AF.Square,
                        accum_out=st4[:, 2 * c + 1 : 2 * c + 2],
                    )
                st2o = work.tile([128, 2], f32, name="st2o", tag="st2o")
                nc.vector.tensor_add(st2o[:], st4[:, 0:2], st4[:, 2:4])

                if i < NLAYERS - 1:
                    # ---- transpose zb -> natural rows, pack AG payload ----
                    rnat = work.tile([128, 8, 128], bf16, name="rnat", tag="rnat")
                    for t in range(RT):
                        ptp = psT.tile([128, 128], bf16, name="ptp", tag="psT")
                        nc.tensor.transpose(
                            ptp[:], zb[:, t * 128 : (t + 1) * 128], idn_sb[:]
                        )
                        nc.vector.tensor_copy(rnat[:, t, :], ptp[:])
                    nc.sync.dma_start(
                        agi[i][0:R, :].rearrange("(k p) c -> p k c", p=128),
                        rnat[:],
                    )
                    nc.scalar.dma_start(
                        agi[i][R : R + 4, :], st2o[:].bitcast(bf16)
                    )
                    nc.gpsimd.collective_compute(
                        "AllGather",
                        mybir.AluOpType.bypass,
                        replica_groups=rg,
                        ins=[agi[i].opt()],
                        outs=[ago[i].opt()],
                    )
                    # reload stationary tiles (8 big DMAs) + gathered stats
                    for g in range(G):
                        nc.sync.dma_start(
                            s_g[g][:],
                            ago[i][
                                g * AGROWS : g * AGROWS + R, :
                            ].rearrange("(k p) c -> p k c", p=128),
                        )
                    gstats = work.tile(
                        [128, 16], f32, name=f"gstats{i}", tag=f"gstats{i}"
                    )
                    for g in range(G):
                        nc.scalar.dma_start(
                            gstats[:, 2 * g : 2 * g + 2].bitcast(bf16),
                            ago[i][g * AGROWS + R : g * AGROWS + R + 4, :],
                        )
                else:
                    # ---- final layer: stats-only AllGather ----------------
                    nc.scalar.dma_start(agi2[:], st2o[:].bitcast(bf16))
                    nc.gpsimd.collective_compute(
                        "AllGather",
                        mybir.AluOpType.bypass,
                        replica_groups=rg,
                        ins=[agi2.opt()],
                        outs=[ago2.opt()],
                    )
                    gs2 = work.tile([128, 16], f32, name="gs2", tag="gs2")
                    for g in range(G):
                        nc.scalar.dma_start(
                            gs2[:, 2 * g : 2 * g + 2].bitcast(bf16),
                            ago2[g * 4 : g * 4 + 4, :],
                        )
                    gsc2 = work.tile([128, 8], f32, name="gsc2", tag="gsc2")
                    fst = work.tile([128, 2], f32, name="fst", tag="fst")
                    nc.vector.tensor_add(gsc2[:], gs2[:, 0:8], gs2[:, 8:16])
                    nc.vector.tensor_add(gsc2[:, 0:4], gsc2[:, 0:4], gsc2[:, 4:8])
                    nc.vector.tensor_add(fst[:], gsc2[:, 0:2], gsc2[:, 2:4])
                    mu2 = work.tile([128, 1], f32, name="mu2", tag="mu2")
                    ex22 = work.tile([128, 1], f32, name="ex22", tag="ex22")
                    var2 = work.tile([128, 1], f32, name="var2", tag="var2")
                    sd2 = work.tile([128, 1], f32, name="sd2", tag="sd2")
                    inv2 = work.tile([128, 1], f32, name="inv2", tag="inv2")
                    aco2 = work.tile([128, 1], f32, name="aco2", tag="aco2")
                    cco2 = work.tile([128, 1], f32, name="cco2", tag="cco2")
                    nc.vector.tensor_scalar_mul(mu2[:], fst[:, 0:1], 1.0 / N)
                    nc.vector.tensor_scalar_mul(ex22[:], fst[:, 1:2], 1.0 / N)
                    nc.vector.tensor_mul(var2[:], mu2[:], mu2[:])
                    nc.vector.tensor_sub(var2[:], ex22[:], var2[:])
                    nc.vector.tensor_scalar_add(var2[:], var2[:], EPS)
                    nc.scalar.sqrt(sd2[:], var2[:])
                    nc.vector.reciprocal(inv2[:], sd2[:])
                    nc.vector.tensor_mul(aco2[:], gamma_sb[:, i : i + 1], inv2[:])
                    nc.vector.tensor_mul(cco2[:], mu2[:], aco2[:])
                    nc.vector.tensor_sub(cco2[:], beta_sb[:, i : i + 1], cco2[:])
                    zaff = work.tile([128, R], bf16, name="zaff", tag="zaff")
                    nc.scalar.activation(
                        zaff[:], zb[:], AF.Identity, bias=cco2[:], scale=aco2[:]
                    )
                    osb = work.tile([128, RT * DOUT], f32, name="osb", tag="osb")
                    for t in range(RT):
                        po = psS.tile([128, DOUT], f32, name="po", tag="psS")
                        nc.tensor.matmul(
                            po[:], ones_sb[:], boutb_sb[:],
                            start=True, stop=False,
                        )
                        nc.tensor.matmul(
                            po[:],
                            zaff[:, t * 128 : (t + 1) * 128],
                            wout_sb[:],
                            start=False, stop=True,
                        )
                        nc.vector.tensor_copy(
                            osb[:, t * DOUT : (t + 1) * DOUT], po[:]
                        )
                        nc.sync.dma_start(
                            out[t * 128 : (t + 1) * 128, :],
                            osb[:, t * DOUT : (t + 1) * DOUT],
                        )

    nc.compile()
    return nc


def _get_module():
    if "nc" not in _cache:
        _cache["nc"] = _build_module()
    return _cache["nc"]


def _prep_inputs(inputs):
    """Host-side sharding / layout prep (transpose + cast + slice + degrees)."""
    x = np.asarray(inputs["x"], np.float32)
    adj = np.asarray(inputs["adj"], np.float32)
    xt = np.ascontiguousarray(x.T).astype(BF16)                   # [128, N]
    bias = np.stack(
        [np.asarray(inputs[f"b{i}"], np.float32) for i in range(NLAYERS)]
    ).reshape(NLAYERS, 128, 1)
    gamma = np.stack(
        [np.asarray(inputs[f"g{i}"], np.float32) for i in range(NLAYERS)]
    ).reshape(NLAYERS, 128, 1)
    beta = np.stack(
        [np.asarray(inputs[f"be{i}"], np.float32) for i in range(NLAYERS)]
    ).reshape(NLAYERS, 128, 1)
    common = {
        "xt": xt,
        "w0": np.asarray(inputs["W0"], np.float32).astype(BF16),
        "w1": np.asarray(inputs["W1"], np.float32).astype(BF16),
        "w2": np.asarray(inputs["W2"], np.float32).astype(BF16),
        "wout": np.asarray(inputs["Wout"], np.float32).astype(BF16),
        "boutb": np.asarray(inputs["bout"], np.float32).reshape(1, DOUT).astype(BF16),
        "idn": np.eye(128, dtype=np.float32).astype(BF16),
        "biasd": bias,
        "gammad": gamma,
        "betad": beta,
    }
    deg = adj.sum(axis=1)                                          # [N]
    in_maps = []
    for c in range(NC):
        rows = slice(c * R, (c + 1) * R)
        adjt_c = np.ascontiguousarray(adj[rows, :].astype(FP8).T)  # [N, R]
        d_c = deg[rows].reshape(1, R).astype(BF16)
        in_maps.append({"adjt": adjt_c, "dd": d_c, **common})
    return in_maps


def run(inputs, trace=False):
    from concourse.bass_utils import run_bass_kernel_spmd

    nc = _get_module()
    in_maps = _prep_inputs(inputs)
    res = run_bass_kernel_spmd(
        nc, in_maps, core_ids=list(range(NC)), trace=trace
    )
    out = np.concatenate(
        [res.results[c]["out"] for c in range(NC)], axis=0
    ).astype(np.float32)
    return out, res


def kernel(**inputs):
    out, _ = run(inputs, trace=False)
    return out


# revision 5
# speedup vs baseline: 1.4168x; 1.0570x over previous
"""Trainium2 Bass kernel for a 3-layer GCN (KnowledgeGraphGNN).

Reference (per layer i):  h = BN_i(relu(adj @ (h @ W_i) + b_i)),  then
out = h @ Wout + bout.

Sharding: nodes row-partitioned over 8 cores.  Each core keeps its adj^T
column block [N, R] resident in SBUF as fp8 (adj is 0/1 -> exact) and the
full "stationary" activation matrix [N, 128] in bf16 k-tiles.  The
aggregation matmul runs in transposed space: P^T [128, R] = S^T @ adjT_c,
64 k-tiles accumulated on the PE (N=512 moving slices).

Collective structure (one AllGather per hidden layer): the *raw* post-ReLU
activations R_i (bf16, natural layout via PE transposes) are gathered
together with the per-core BN partial sums packed into the same buffer.
The BN affine is folded into the next layer algebraically:

    h_{i+1} = adj @ (BN_i(R_i) @ W) = (adj @ R_i) @ diag(a_i) W  +  d x r_i

with a_i = gamma_i/sigma_i, r_i = (beta_i - mu_i a_i) @ W, and d = adj @ 1
the degree vector (shipped as adjacency metadata).  The rank-1 d x r term
is seeded into PSUM with a K=1 outer-product matmul before the W matmuls
accumulate on top.  The final layer needs only a tiny stats AllGather.
"""

import numpy as np
import ml_dtypes

BF16 = ml_dtypes.bfloat16
FP8 = ml_dtypes.float8_e4m3

N = 8192          # nodes
DH = 128          # hidden dim (= partition count)
DOUT = 64
NC = 8            # cores
R = N // NC       # rows per core = 1024
KT = N // 128     # contraction k-tiles = 64
G = 8             # k-tile groups (8 tiles each)
RT = R // 128     # node tiles per core = 8
NLAYERS = 3
EPS = 1e-5
AGROWS = R + 4    # AG payload rows: R activation rows + 4 rows (=1KB) stats

_cache = {}


def _build_module():
    from concourse import bacc, tile
    import concourse.mybir as mybir

    f32 = mybir.dt.float32
    bf16 = mybir.dt.bfloat16
    fp8 = mybir.dt.float8e4
    AF = mybir.ActivationFunctionType

    nc = bacc.Bacc(None, target_bir_lowering=False, num_devices=NC)

    # ---- kernel I/O --------------------------------------------------------
    adjt = nc.dram_tensor("adjt", [N, R], fp8, kind="ExternalInput")
    xt = nc.dram_tensor("xt", [128, N], bf16, kind="ExternalInput")
    w0 = nc.dram_tensor("w0", [128, 128], bf16, kind="ExternalInput")
    w1 = nc.dram_tensor("w1", [128, 128], bf16, kind="ExternalInput")
    w2 = nc.dram_tensor("w2", [128, 128], bf16, kind="ExternalInput")
    wout = nc.dram_tensor("wout", [128, DOUT], bf16, kind="ExternalInput")
    boutb = nc.dram_tensor("boutb", [1, DOUT], bf16, kind="ExternalInput")
    dd = nc.dram_tensor("dd", [1, R], bf16, kind="ExternalInput")
    idn = nc.dram_tensor("idn", [128, 128], bf16, kind="ExternalInput")
    biasd = nc.dram_tensor("biasd", [NLAYERS, 128, 1], f32, kind="ExternalInput")
    gammad = nc.dram_tensor("gammad", [NLAYERS, 128, 1], f32, kind="ExternalInput")
    betad = nc.dram_tensor("betad", [NLAYERS, 128, 1], f32, kind="ExternalInput")
    out = nc.dram_tensor("out", [R, DOUT], f32, kind="ExternalOutput")

    rg = [list(range(NC))]
    wdram = (w0, w1, w2)

    with tile.TileContext(nc) as tc:
        with (
            tc.tile_pool(name="const", bufs=1) as const,
            tc.tile_pool(name="adjp", bufs=1) as adjp,
            tc.tile_pool(name="sp", bufs=1) as sp,
            tc.tile_pool(name="work", bufs=1) as work,
            tc.tile_pool(name="psA", bufs=1, space="PSUM") as psA,
            tc.tile_pool(name="psH", bufs=1, space="PSUM") as psH,
            tc.tile_pool(name="psS", bufs=2, space="PSUM") as psS,
            tc.tile_pool(name="psT", bufs=2, space="PSUM") as psT,
            tc.tile_pool(name="dram", bufs=1, space="DRAM") as dram,
        ):
            # ---- constants (scalar engine issues these tiny DMAs) ----------
            w_sb = []
            for i in range(NLAYERS):
                t = const.tile([128, 128], bf16, name=f"w{i}_sb", tag=f"w{i}_sb")
                nc.scalar.dma_start(t[:], wdram[i][:])
                w_sb.append(t)
            wout_sb = const.tile([128, DOUT], bf16, name="wout_sb")
            nc.scalar.dma_start(wout_sb[:], wout[:])
            boutb_sb = const.tile([1, DOUT], bf16, name="boutb_sb")
            nc.scalar.dma_start(boutb_sb[:], boutb[:])
            ones_sb = const.tile([1, 128], bf16, name="ones_sb")
            nc.vector.memset(ones_sb[:], 1.0)
            d_sb = const.tile([1, R], bf16, name="d_sb")
            nc.scalar.dma_start(d_sb[:], dd[:])
            idn_sb = const.tile([128, 128], bf16, name="idn_sb")
            nc.scalar.dma_start(idn_sb[:], idn[:])
            bias_sb = const.tile([128, NLAYERS], f32, name="bias_sb")
            gamma_sb = const.tile([128, NLAYERS], f32, name="gamma_sb")
            beta_sb = const.tile([128, NLAYERS], f32, name="beta_sb")
            for i in range(NLAYERS):
                nc.scalar.dma_start(bias_sb[:, i : i + 1], biasd[i])
                nc.scalar.dma_start(gamma_sb[:, i : i + 1], gammad[i])
                nc.scalar.dma_start(beta_sb[:, i : i + 1], betad[i])

            # x^T first so the layer-0 stationary build can start immediately
            xt_sb = const.tile([128, N], bf16, name="xt_sb")
            nc.sync.dma_start(xt_sb[:], xt[:])

            # ---- adj^T resident in SBUF: 8 group tiles, 1 DMA each ---------
            adj_g = []
            for g in range(G):
                t = adjp.tile([128, 8, R], fp8, name=f"adj_{g}", tag=f"adj_{g}")
                src = adjt[g * 1024 : (g + 1) * 1024, :].rearrange(
                    "(k p) c -> p k c", p=128
                )
                nc.gpsimd.dma_start(t[:], src)
                adj_g.append(t)

            def adj_mv(k, lo, size):
                g, sub = divmod(k, 8)
                return adj_g[g][:, sub, lo : lo + size]

            # ---- stationary activation tiles (8 groups of 8 k-tiles) -------
            s_g = [
                sp.tile([128, 8, 128], bf16, name=f"s_{g}", tag=f"s_{g}")
                for g in range(G)
            ]

            def s_tile(k):
                g, sub = divmod(k, 8)
                return s_g[g][:, sub, :]

            # Layer 0 stationary: S0 = x @ W0, built locally on every core.
            for k in range(KT):
                ps0 = psS.tile([128, 128], f32, name="ps0", tag="psS")
                nc.tensor.matmul(ps0[:], xt_sb[:, k * 128 : (k + 1) * 128], w_sb[0][:])
                nc.vector.tensor_copy(s_tile(k), ps0[:])

            # per-layer DRAM comm tiles
            agi = [
                dram.tile([AGROWS, 128], bf16, name=f"agi{i}", tag=f"agi{i}")
                for i in range(2)
            ]
            ago = [
                dram.tile(
                    [NC * AGROWS, 128], bf16, name=f"ago{i}", tag=f"ago{i}",
                    addr_space="Shared",
                )
                for i in range(2)
            ]
            agi2 = dram.tile([4, 128], bf16, name="agi2", tag="agi2")
            ago2 = dram.tile([32, 128], bf16, name="ago2", tag="ago2",
                             addr_space="Shared")

            gstats = None  # SBUF tile holding the 8 gathered stat blocks

            for i in range(NLAYERS):
                # ---- A: P^T [128, R] = S^T @ adjT_c  (64 k-tiles, N=512) --
                if i == 0:
                    ph = psH.tile([128, R], f32, name="ph", tag="ph")
                    for nch in range(R // 512):
                        lo = nch * 512
                        for k in range(KT):
                            nc.tensor.matmul(
                                ph[:, lo : lo + 512],
                                s_tile(k),
                                adj_mv(k, lo, 512),
                                start=(k == 0),
                                stop=(k == KT - 1),
                            )
                else:
                    pa = psA.tile([128, R], f32, name="pa", tag="pa")
                    for nch in range(R // 512):
                        lo = nch * 512
                        for k in range(KT):
                            nc.tensor.matmul(
                                pa[:, lo : lo + 512],
                                s_tile(k),
                                adj_mv(k, lo, 512),
                                start=(k == 0),
                                stop=(k == KT - 1),
                            )
                    pm = work.tile([128, R], f32, name="pm", tag="pm")
                    nc.vector.tensor_copy(pm[:], pa[:])

                    # stats of layer i-1 arrived inside AG i-1: combine them.
                    gsc = work.tile([128, 8], f32, name="gsc", tag="gsc")
                    st2 = work.tile([128, 2], f32, name="st2", tag="st2")
                    nc.vector.tensor_add(gsc[:], gstats[:, 0:8], gstats[:, 8:16])
                    nc.vector.tensor_add(gsc[:, 0:4], gsc[:, 0:4], gsc[:, 4:8])
                    nc.vector.tensor_add(st2[:], gsc[:, 0:2], gsc[:, 2:4])
                    # mu, var, a = gamma/sigma, c = beta - mu*a
                    mu = work.tile([128, 1], f32, name="mu", tag="mu")
                    ex2 = work.tile([128, 1], f32, name="ex2", tag="ex2")
                    var = work.tile([128, 1], f32, name="var", tag="var")
                    sd = work.tile([128, 1], f32, name="sd", tag="sd")
                    inv = work.tile([128, 1], f32, name="inv", tag="inv")
                    aco = work.tile([128, 1], f32, name="aco", tag="aco")
                    cco = work.tile([128, 1], f32, name="cco", tag="cco")
                    ccb = work.tile([128, 1], bf16, name="ccb", tag="ccb")
                    nc.vector.tensor_scalar_mul(mu[:], st2[:, 0:1], 1.0 / N)
                    nc.vector.tensor_scalar_mul(ex2[:], st2[:, 1:2], 1.0 / N)
                    nc.vector.tensor_mul(var[:], mu[:], mu[:])
                    nc.vector.tensor_sub(var[:], ex2[:], var[:])
                    nc.vector.tensor_scalar_add(var[:], var[:], EPS)
                    nc.scalar.sqrt(sd[:], var[:])
                    nc.vector.reciprocal(inv[:], sd[:])
                    nc.vector.tensor_mul(aco[:], gamma_sb[:, i - 1 : i], inv[:])
                    nc.vector.tensor_mul(cco[:], mu[:], aco[:])
                    nc.vector.tensor_sub(cco[:], beta_sb[:, i - 1 : i], cco[:])
                    nc.vector.tensor_copy(ccb[:], cco[:])
                    # Wa = diag(a) @ W_i  (bf16), r = c @ W_i  (bf16 row)
                    wa = work.tile([128, 128], f32, name="wa", tag="wa")
                    nc.scalar.activation(wa[:], w_sb[i][:], AF.Copy, scale=aco[:])
                    pr = psS.tile([1, 128], f32, name="pr", tag="psS")
                    nc.tensor.matmul(pr[:], ccb[:], w_sb[i][:])
                    rrow = work.tile([1, 128], bf16, name="rrow", tag="rrow")
                    nc.vector.tensor_copy(rrow[:], pr[:])

                    # ---- transform: ph = Wa^T @ Pm + outer(r, d) ----------
                    ph = psH.tile([128, R], f32, name="ph", tag="ph")
                    for nch in range(R // 512):
                        lo = nch * 512
                        nc.tensor.matmul(
                            ph[:, lo : lo + 512],
                            rrow[:],
                            d_sb[:, lo : lo + 512],
                            start=True, stop=False,
                        )
                        nc.tensor.matmul(
                            ph[:, lo : lo + 512],
                            wa[:],
                            pm[:, lo : lo + 512],
                            start=False, stop=True,
                        )

                # ---- B/C: zb = relu(ph + b_i) (bf16) + partial stats ------
                zb = work.tile([128, R], bf16, name="zb", tag="zb")
                sq = work.tile([128, R], f32, name="sq", tag="sq")
                st4 = work.tile([128, 4], f32, name="st4", tag="st4")
                for c in range(2):
                    lo = c * 512
                    nc.scalar.activation(
                        zb[:, lo : lo + 512],
                        ph[:, lo : lo + 512],
                        AF.Relu,
                        bias=bias_sb[:, i : i + 1],
                        scale=1.0,
                        accum_out=st4[:, 2 * c : 2 * c + 1],
                    )
                    nc.scalar.activation(
                        sq[:, lo : lo + 512],
                        zb[:, lo : lo + 512],
                        AF.Square,
                        accum_out=st4[:, 2 * c + 1 : 2 * c + 2],
                    )
                st2o = work.tile([128, 2], f32, name="st2o", tag="st2o")
                nc.vector.tensor_add(st2o[:], st4[:, 0:2], st4[:, 2:4])

                if i < NLAYERS - 1:
                    # ---- transpose zb -> natural rows, pack AG payload ----
                    rnat = work.tile([128, 8, 128], bf16, name="rnat", tag="rnat")
                    for t in range(RT):
                        ptp = psT.tile([128, 128], bf16, name="ptp", tag="psT")
                        nc.tensor.transpose(
                            ptp[:], zb[:, t * 128 : (t + 1) * 128], idn_sb[:]
                        )
                        nc.vector.tensor_copy(rnat[:, t, :], ptp[:])
                    nc.sync.dma_start(
                        agi[i][0:R, :].rearrange("(k p) c -> p k c", p=128),
                        rnat[:],
                    )
                    nc.scalar.dma_start(
                        agi[i][R : R + 4, :], st2o[:].bitcast(bf16)
                    )
                    nc.gpsimd.collective_compute(
                        "AllGather",
                        mybir.AluOpType.bypass,
                        replica_groups=rg,
                        ins=[agi[i].opt()],
                        outs=[ago[i].opt()],
                    )
                    # reload stationary tiles (8 big DMAs) + gathered stats
                    for g in range(G):
                        nc.sync.dma_start(
                            s_g[g][:],
                            ago[i][
                                g * AGROWS : g * AGROWS + R, :
                            ].rearrange("(k p) c -> p k c", p=128),
                        )
                    gstats = work.tile(
                        [128, 16], f32, name=f"gstats{i}", tag=f"gstats{i}"
                    )
                    for g in range(G):
                        nc.scalar.dma_start(
                            gstats[:, 2 * g : 2 * g + 2].bitcast(bf16),
                            ago[i][g * AGROWS + R : g * AGROWS + R + 4, :],
                        )
                else:
                    # ---- final layer: stats-only AllGather ----------------
                    nc.scalar.dma_start(agi2[:], st2o[:].bitcast(bf16))
                    nc.gpsimd.collective_compute(
                        "AllGather",
                        mybir.AluOpType.bypass,
                        replica_groups=rg,
                        ins=[agi2.opt()],
                        outs=[ago2.opt()],
                    )
                    gs2 = work.tile([128, 16], f32, name="gs2", tag="gs2")
                    for g in range(G):
                        nc.scalar.dma_start(
                            gs2[:, 2 * g : 2 * g + 2].bitcast(bf16),
                            ago2[g * 4 : g * 4 + 4, :],
                        )
                    gsc2 = work.tile([128, 8], f32, name="gsc2", tag="gsc2")
                    fst = work.tile([128, 2], f32, name="fst", tag="fst")
                    nc.vector.tensor_add(gsc2[:], gs2[:, 0:8], gs2[:, 8:16])
                    nc.vector.tensor_add(gsc2[:, 0:4], gsc2[:, 0:4], gsc2[:, 4:8])
                    nc.vector.tensor_add(fst[:], gsc2[:, 0:2], gsc2[:, 2:4])
                    mu2 = work.tile([128, 1], f32, name="mu2", tag="mu2")
                    ex22 = work.tile([128, 1], f32, name="ex22", tag="ex22")
                    var2 = work.tile([128, 1], f32, name="var2", tag="var2")
                    sd2 = work.tile([128, 1], f32, name="sd2", tag="sd2")
                    inv2 = work.tile([128, 1], f32, name="inv2", tag="inv2")
                    aco2 = work.tile([128, 1], f32, name="aco2", tag="aco2")
                    cco2 = work.tile([128, 1], f32, name="cco2", tag="cco2")
                    nc.vector.tensor_scalar_mul(mu2[:], fst[:, 0:1], 1.0 / N)
                    nc.vector.tensor_scalar_mul(ex22[:], fst[:, 1:2], 1.0 / N)
                    nc.vector.tensor_mul(var2[:], mu2[:], mu2[:])
                    nc.vector.tensor_sub(var2[:], ex22[:], var2[:])
                    nc.vector.tensor_scalar_add(var2[:], var2[:], EPS)
                    nc.scalar.sqrt(sd2[:], var2[:])
                    nc.vector.reciprocal(inv2[:], sd2[:])
                    nc.vector.tensor_mul(aco2[:], gamma_sb[:, i : i + 1], inv2[:])
                    nc.vector.tensor_mul(cco2[:], mu2[:], aco2[:])
                    nc.vector.tensor_sub(cco2[:], beta_sb[:, i : i + 1], cco2[:])
                    zaff = work.tile([128, R], bf16, name="zaff", tag="zaff")
                    nc.scalar.activation(
                        zaff[:], zb[:], AF.Identity, bias=cco2[:], scale=aco2[:]
                    )
                    osb = work.tile([128, RT * DOUT], f32, name="osb", tag="osb")
                    for t in range(RT):
                        po = psS.tile([128, DOUT], f32, name="po", tag="psS")
                        nc.tensor.matmul(
                            po[:], ones_sb[:], boutb_sb[:],
                            start=True, stop=False,
                        )
                        nc.tensor.matmul(
                            po[:],
                            zaff[:, t * 128 : (t + 1) * 128],
                            wout_sb[:],
                            start=False, stop=True,
                        )
                        nc.vector.tensor_copy(
                            osb[:, t * DOUT : (t + 1) * DOUT], po[:]
                        )
                        nc.sync.dma_start(
                            out[t * 128 : (t + 1) * 128, :],
                            osb[:, t * DOUT : (t + 1) * DOUT],
                        )

    nc.compile()
    return nc


def _get_module():
    if "nc" not in _cache:
        _cache["nc"] = _build_module()
    return _cache["nc"]


def _prep_inputs(inputs):
    """Host-side sharding / layout prep (transpose + cast + slice + degrees)."""
    x = np.asarray(inputs["x"], np.float32)
    adj = np.asarray(inputs["adj"], np.float32)
    xt = np.ascontiguousarray(x.T).astype(BF16)                   # [128, N]
    bias = np.stack(
        [np.asarray(inputs[f"b{i}"], np.float32) for i in range(NLAYERS)]
    ).reshape(NLAYERS, 128, 1)
    gamma = np.stack(
        [np.asarray(inputs[f"g{i}"], np.float32) for i in range(NLAYERS)]
    ).reshape(NLAYERS, 128, 1)
    beta = np.stack(
        [np.asarray(inputs[f"be{i}"], np.float32) for i in range(NLAYERS)]
    ).reshape(NLAYERS, 128, 1)
    common = {
        "xt": xt,
        "w0": np.asarray(inputs["W0"], np.float32).astype(BF16),
        "w1": np.asarray(inputs["W1"], np.float32).astype(BF16),
        "w2": np.asarray(inputs["W2"], np.float32).astype(BF16),
        "wout": np.asarray(inputs["Wout"], np.float32).astype(BF16),
        "boutb": np.asarray(inputs["bout"], np.float32).reshape(1, DOUT).astype(BF16),
        "idn": np.eye(128, dtype=np.float32).astype(BF16),
        "biasd": bias,
        "gammad": gamma,
        "betad": beta,
    }
    deg = adj.sum(axis=1)                                          # [N]
    in_maps = []
    for c in range(NC):
        rows = slice(c * R, (c + 1) * R)
        adjt_c = np.ascontiguousarray(adj[rows, :].astype(FP8).T)  # [N, R]
        d_c = deg[rows].reshape(1, R).astype(BF16)
        in_maps.append({"adjt": adjt_c, "dd": d_c, **common})
    return in_maps


def run(inputs, trace=False):
    from concourse.bass_utils import run_bass_kernel_spmd

    nc = _get_module()
    in_maps = _prep_inputs(inputs)
    res = run_bass_kernel_spmd(
        nc, in_maps, core_ids=list(range(NC)), trace=trace
    )
    out = np.concatenate(
        [res.results[c]["out"] for c in range(NC)], axis=0
    ).astype(np.float32)
    return out, res


def kernel(**inputs):
    out, _ = run(inputs, trace=False)
    return out
